# revision 1
# baseline (speedup 1.0000x reference)
"""Multi-head causal attention on 8 TRN2 NeuronCores.

Sharding: tensor-parallel over heads — 16 heads / 8 cores = 2 heads per core.
Each core computes q/k/v projections for its 2 heads (column-sharded QKV
weights), causal attention for those heads over both batch elements, and the
row-sharded slice of the output projection, producing a full-shape partial
output.  Host sums the 8 partials and adds bo + bv @ Wo.T (the per-head value
bias commutes through the output projection because attention rows sum to 1).

All matmuls run in float32r (full-rate fp32 on the PE array, ~1e-4 rel).
Layout/structure choices:
  - x is passed transposed (E-major) so QKV matmuls need no on-device
    transposes; weights are host-transposed likewise
  - scores are computed transposed [k, q] so the attn @ v matmul's operands
    arrive in exactly the layout the PE wants; softmax denominators come from
    a ones-column matmul accumulated alongside
  - diagonal k-tiles only compute/accumulate their causally valid column
    sub-range (exact: diagonal tiles are last in each k-loop)
  - output projection is fused into the attention q-tile loop; evacuation
    copies alternate between DVE and ACT; PSUM tags are budgeted to 8 banks
    (proj/out-proj 2, scores 4, attn accumulator 1, denominators 1)
"""

import sys

if "/opt/trn_rl_repo" not in sys.path:
    sys.path.insert(0, "/opt/trn_rl_repo")

import numpy as np

import concourse.bass as bass  # noqa: F401  (engine namespaces live on nc)
import concourse.tile as tile
from concourse import bacc, mybir
from concourse.bass_utils import run_bass_kernel_spmd

F32 = mybir.dt.float32
F32R = mybir.dt.float32r
AF = mybir.ActivationFunctionType
ALU = mybir.AluOpType

B, S, E = 2, 2048, 2048
H, D = 16, 128
NCORES = 8
HPC = H // NCORES          # heads per core = 2
M = HPC * D                # local channels per core = 256
EO = E // 128              # 16 contraction chunks
XT = 256                   # token-tile width for projections
NT = S // XT               # 8 token tiles per batch
QT = 512                   # q-tile width for attention
NQT = S // QT              # 4 q-tiles
ET = 512                   # e-tile width for out-projection
SCALE = 1.0 / float(np.sqrt(D))
MASK_BIAS = -30.0


def build_nc():
    nc = bacc.Bacc(trn_type="TRN2", target_bir_lowering=False, num_swdge_queues=4)

    xT = nc.declare_dram_parameter("xT", [B, E, S], F32, isOutput=False)
    wq = nc.declare_dram_parameter("wq", [E, M], F32, isOutput=False)
    wk = nc.declare_dram_parameter("wk", [E, M], F32, isOutput=False)
    wv = nc.declare_dram_parameter("wv", [E, M], F32, isOutput=False)
    wo = nc.declare_dram_parameter("wo", [M, E], F32, isOutput=False)
    bq = nc.declare_dram_parameter("bq", [128, HPC], F32, isOutput=False)
    bk = nc.declare_dram_parameter("bk", [128, HPC], F32, isOutput=False)
    tb = nc.declare_dram_parameter("tb", [128, 128], F32, isOutput=False)
    on = nc.declare_dram_parameter("on", [128, 1], F32, isOutput=False)
    o = nc.declare_dram_parameter("o", [B, S, E], F32, isOutput=True)

    with tile.TileContext(nc) as tc:
        _body(tc, nc, xT, wq, wk, wv, wo, bq, bk, tb, on, o)
    nc.compile()
    return nc


def _body(tc, nc, xT, wq, wk, wv, wo, bq, bk, tb, on, o):
    from contextlib import ExitStack

    ctx = ExitStack()
    with ctx:
        wpool = ctx.enter_context(tc.tile_pool(name="w", bufs=1))
        xpool = ctx.enter_context(tc.tile_pool(name="x", bufs=2))
        qkv = ctx.enter_context(tc.tile_pool(name="qkv", bufs=1))
        epool = ctx.enter_context(tc.tile_pool(name="e", bufs=6))
        otp = ctx.enter_context(tc.tile_pool(name="ot", bufs=1))
        osp = ctx.enter_context(tc.tile_pool(name="os", bufs=8))
        rp = ctx.enter_context(tc.tile_pool(name="r", bufs=2))
        psA = ctx.enter_context(tc.tile_pool(name="psA", bufs=2, space="PSUM"))
        psC = ctx.enter_context(tc.tile_pool(name="psC", bufs=4, space="PSUM"))
        psB1 = ctx.enter_context(tc.tile_pool(name="psB1", bufs=1, space="PSUM"))

        # ---- weights / constants (once) ----
        wq_sb = wpool.tile([128, EO, M], F32R, tag="wq")
        wk_sb = wpool.tile([128, EO, M], F32R, tag="wk")
        wv_sb = wpool.tile([128, EO, M], F32R, tag="wv")
        wo_sb = wpool.tile([128, HPC, E], F32R, tag="wo")
        on_sb = wpool.tile([128, 1], F32R, tag="on")
        nc.gpsimd.dma_start(on_sb[:], on[:])
        # Warm the PE (HAM clock gate) with tiny matmuls while x0/weights load.
        warm_rhs = rp.tile([128, 512], F32, tag="rb")
        nc.vector.memset(warm_rhs[:], 0.0)
        warm = psC.tile([128, 512], F32, tag="sc")
        for _ in range(24):
            nc.tensor.matmul(warm[:1, :], on_sb[:], warm_rhs[:].bitcast(F32R),
                             start=True, stop=True)
        # x tile 0 and wq stream in interleaved chunks so the first projection
        # matmuls start as soon as their first contraction chunks land; wo is
        # not needed until attention output, so it loads last.
        x_first = xpool.tile([128, EO, XT], F32R, tag="x")
        _xr0 = xT[0].rearrange("(eo p) s -> p eo s", p=128)
        _wqr = wq.rearrange("(eo p) m -> p eo m", p=128)
        for lo, hi in ((0, 4), (4, 8), (8, 16)):
            nc.gpsimd.dma_start(x_first[:, lo:hi], _xr0[:, lo:hi, 0:XT])
            nc.gpsimd.dma_start(wq_sb[:, lo:hi], _wqr[:, lo:hi])
        nc.gpsimd.dma_start(wk_sb[:], wk.rearrange("(eo p) m -> p eo m", p=128))
        nc.gpsimd.dma_start(wv_sb[:], wv.rearrange("(eo p) m -> p eo m", p=128))
        x_second = xpool.tile([128, EO, XT], F32R, tag="x")
        nc.gpsimd.dma_start(
            x_second[:], xT[0].rearrange("(eo p) s -> p eo s", p=128)[:, :, XT:2 * XT]
        )
        nc.gpsimd.dma_start(wo_sb[:], wo.rearrange("(h p) e -> p h e", p=128))
        bq_sb = wpool.tile([128, HPC], F32, tag="bq")
        bk_sb = wpool.tile([128, HPC], F32, tag="bk")
        nc.sync.dma_start(bq_sb[:], bq[:])
        nc.sync.dma_start(bk_sb[:], bk[:])
        tb_sb = wpool.tile([128, 128], F32, tag="tb")
        nc.sync.dma_start(tb_sb[:], tb[:])


        for b in range(B):
            # ---- Q/K/V projections for batch b ----
            qT_sb = qkv.tile([128, HPC, S], F32R, tag="qT")
            kT_sb = qkv.tile([128, HPC, S], F32R, tag="kT")
            v_sb = qkv.tile([128, S // 128, M], F32R, tag="v")
            oT_sb = otp.tile([128, HPC, S], F32R, tag="oT")
            qT_h = [qT_sb[:, h] for h in range(HPC)]
            kT_h = [kT_sb[:, h] for h in range(HPC)]
            v_h = [v_sb[:, :, h * D:(h + 1) * D] for h in range(HPC)]
            oT_h = [oT_sb[:, h] for h in range(HPC)]
            xTb = xT[b].rearrange("(eo p) s -> p eo s", p=128)

            def proj_tile(t, x_t):
                for h in range(HPC):
                    for w_sb, dsts, bias, scl in (
                        (wq_sb, qT_h, bq_sb, SCALE),
                        (wk_sb, kT_h, bk_sb, 1.0),
                    ):
                        ps = psA.tile([128, 512], F32, tag="qkv")
                        for eo in range(EO):
                            nc.tensor.matmul(
                                ps[:, :XT],
                                w_sb[:, eo, h * D:(h + 1) * D],
                                x_t[:, eo, :],
                                start=(eo == 0),
                                stop=(eo == EO - 1),
                            )
                        nc.scalar.activation(
                            dsts[h][:, t * XT:(t + 1) * XT],
                            ps[:, :XT],
                            AF.Identity,
                            bias=bias[:, h:h + 1],
                            scale=scl,
                        )
                for st in range(XT // 128):
                    ps = psA.tile([128, 512], F32, tag="qkv")
                    for eo in range(EO):
                        nc.tensor.matmul(
                            ps[:, :M],
                            x_t[:, eo, st * 128:(st + 1) * 128],
                            wv_sb[:, eo, :],
                            start=(eo == 0),
                            stop=(eo == EO - 1),
                        )
                    nc.vector.tensor_copy(
                        v_sb[:, t * (XT // 128) + st, :], ps[:, :M]
                    )

            def attn_qtile(h, qt):
                q_rhs = qT_h[h][:, qt * QT:(qt + 1) * QT]
                ut = psB1.tile([128, 512], F32, tag="ut")
                sums = psB1.tile([1, 512], F32, tag="sums")
                nkt = (qt + 1) * (QT // 128)
                for kt in range(nkt):
                    jj = kt - qt * (QT // 128)
                    # columns < jj*128 of this k-tile's block are causally
                    # masked; diagonal tiles come last in the k-loop, so
                    # accumulating only the valid sub-range is exact.
                    lo = max(jj, 0) * 128
                    sc = psC.tile([128, 512], F32, tag="sc")
                    nc.tensor.matmul(
                        sc[:, lo:],
                        kT_h[h][:, kt * 128:(kt + 1) * 128],
                        q_rhs[:, lo:],
                        start=True,
                        stop=True,
                    )
                    e = epool.tile([128, 512], F32R, tag="e")
                    if jj >= 0:
                        nc.vector.tensor_tensor(
                            sc[:, jj * 128:(jj + 1) * 128],
                            sc[:, jj * 128:(jj + 1) * 128],
                            tb_sb[:],
                            ALU.add,
                        )
                        if jj > 0:
                            nc.vector.memset(e[:, :lo].bitcast(F32), 0.0)
                        nc.scalar.activation(e[:, lo:], sc[:, lo:], AF.Exp)
                    else:
                        nc.scalar.activation(e[:], sc[:], AF.Exp)
                    nc.tensor.matmul(
                        ut[:, lo:],
                        v_h[h][:, kt, :],
                        e[:, lo:],
                        start=(kt == 0),
                        stop=(kt == nkt - 1),
                    )
                    nc.tensor.matmul(
                        sums[:, lo:],
                        on_sb[:],
                        e[:, lo:],
                        start=(kt == 0),
                        stop=(kt == nkt - 1),
                    )
                rec = rp.tile([1, 512], F32, tag="rec")
                nc.vector.reciprocal(rec[:], sums[:])
                rb = rp.tile([128, 512], F32, tag="rb")
                nc.gpsimd.partition_broadcast(rb[:], rec[:])
                nc.vector.tensor_tensor(
                    oT_h[h][:, qt * QT:(qt + 1) * QT], ut[:], rb[:], ALU.mult
                )

            def out_proj(qt):
                for qi4 in range(QT // 128):
                    qi = qt * (QT // 128) + qi4
                    for et in range(E // ET):
                        ps = psA.tile([128, 512], F32, tag="qkv")
                        for h in range(HPC):
                            nc.tensor.matmul(
                                ps[:],
                                oT_h[h][:, qi * 128:(qi + 1) * 128],
                                wo_sb[:, h, et * ET:(et + 1) * ET],
                                start=(h == 0),
                                stop=(h == HPC - 1),
                            )
                        osb = osp.tile([128, 512], F32, tag="osb")
                        if (qi * (E // ET) + et) % 2 == 0:
                            nc.vector.tensor_copy(osb[:], ps[:])
                        else:
                            nc.scalar.copy(osb[:], ps[:])
                        nc.sync.dma_start(
                            o[b, qi * 128:(qi + 1) * 128, et * ET:(et + 1) * ET],
                            osb[:],
                        )

            for t in range(NT):
                if b == 0 and t == 0:
                    x_t = x_first
                elif b == 0 and t == 1:
                    x_t = x_second
                else:
                    x_t = xpool.tile([128, EO, XT], F32R, tag="x")
                    nc.gpsimd.dma_start(x_t[:], xTb[:, :, t * XT:(t + 1) * XT])
                proj_tile(t, x_t)
            qts = list(range(NQT)) if b == 0 else list(reversed(range(NQT)))
            for qt in qts:
                attn_qtile(0, qt)
                attn_qtile(1, qt)
                out_proj(qt)


_NC_CACHE = None


def _get_nc():
    global _NC_CACHE
    if _NC_CACHE is None:
        _NC_CACHE = build_nc()
    return _NC_CACHE


def _prep_inputs(x, Wq, bq, Wk, bk, Wv, bv, Wo, bo):
    x = np.ascontiguousarray(np.asarray(x, dtype=np.float32))
    xT = np.ascontiguousarray(x.transpose(0, 2, 1))
    tb_np = np.where(
        np.arange(128)[:, None] <= np.arange(128)[None, :], 0.0, MASK_BIAS
    ).astype(np.float32)
    on_np = np.ones((128, 1), dtype=np.float32)
    in_maps = []
    for c in range(NCORES):
        sl = slice(c * M, (c + 1) * M)
        in_maps.append({
            "xT": xT,
            "wq": np.ascontiguousarray(np.asarray(Wq)[sl, :].T.astype(np.float32)),
            "wk": np.ascontiguousarray(np.asarray(Wk)[sl, :].T.astype(np.float32)),
            "wv": np.ascontiguousarray(np.asarray(Wv)[sl, :].T.astype(np.float32)),
            "wo": np.ascontiguousarray(np.asarray(Wo)[:, sl].T.astype(np.float32)),
            "bq": np.ascontiguousarray(
                (np.asarray(bq)[sl].astype(np.float32) * SCALE).reshape(HPC, 128).T
            ),
            "bk": np.ascontiguousarray(
                np.asarray(bk)[sl].astype(np.float32).reshape(HPC, 128).T
            ),
            "tb": tb_np,
            "on": on_np,
        })
    return in_maps


def run(inputs, trace=False):
    in_maps = _prep_inputs(
        inputs["x"], inputs["Wq"], inputs["bq"], inputs["Wk"], inputs["bk"],
        inputs["Wv"], inputs["bv"], inputs["Wo"], inputs["bo"],
    )
    nc = _get_nc()
    res = run_bass_kernel_spmd(nc, in_maps, list(range(NCORES)), trace=trace)
    acc = np.zeros((B, S, E), dtype=np.float64)
    for r in res.results:
        acc += r["o"].astype(np.float64)
    acc += np.asarray(inputs["bo"], dtype=np.float64)[None, None, :]
    acc += (np.asarray(inputs["bv"], dtype=np.float64)
            @ np.asarray(inputs["Wo"], dtype=np.float64).T)[None, None, :]
    return acc.astype(np.float32), res


def kernel(**inputs):
    out, _ = run(inputs, trace=False)
    return out



# revision 25
# speedup vs baseline: 1.1432x; 1.1432x over previous
"""Multi-head causal attention on 8 TRN2 NeuronCores.

Sharding: tensor-parallel over heads — 16 heads / 8 cores = 2 heads per core.
Each core computes q/k/v projections for its 2 heads (column-sharded QKV
weights), causal attention for those heads over both batch elements, and the
row-sharded slice of the output projection, producing a full-shape partial
output.  Host sums the 8 partials and adds bo + bv @ Wo.T (the per-head value
bias commutes through the output projection because attention rows sum to 1).

v2 layout/structure choices (vs the fp32r baseline):
  - fp16 operands everywhere (x, weights, q/k/v, e, oT, output partials);
    PSUM accumulation stays fp32.  fp16 matmuls run 1 cycle/row at any
    width (fp32r needs >=256-wide), halving DMA and SBUF on the way.
  - softmax denominators are NOT computed with ones-row matmuls (those cost
    a full PE pass per e-tile).  Instead, per 128-wide q-chunk, a tiny
    transposed matmul (lhsT = e-chunk, rhs = ones column, output [128,1])
    accumulates Z in PSUM across k-tiles at ~zero PE cost, directly in
    q-partition layout.  One cheap PE transpose per (h,qt) brings 1/Z back
    to row layout for the broadcast + normalization multiply.
  - scores are computed transposed [k, q]; diagonal k-tiles only compute
    their causally valid column sub-range (exact: diagonal tiles are last)
  - engines execute their instruction streams in order, so batch-1
    projections are EMITTED interleaved into batch-0's attention loop to
    fill PE stalls; out-projections in batch 1 are deferred by one q-tile
    to hide the softmax normalization chain.
  - out-projection evacuations round-robin over DVE/ACT/gpsimd.
"""

import sys

if "/opt/trn_rl_repo" not in sys.path:
    sys.path.insert(0, "/opt/trn_rl_repo")

import numpy as np

import concourse.bass as bass  # noqa: F401  (engine namespaces live on nc)
import concourse.tile as tile
from concourse import bacc, mybir
from concourse.bass_utils import run_bass_kernel_spmd

F32 = mybir.dt.float32
F32R = mybir.dt.float32r
F16 = mybir.dt.float16
AF = mybir.ActivationFunctionType
ALU = mybir.AluOpType

B, S, E = 2, 2048, 2048
H, D = 16, 128
NCORES = 8
HPC = H // NCORES          # heads per core = 2
M = HPC * D                # local channels per core = 256
EO = E // 128              # 16 contraction chunks
XT = 256                   # token-tile width for projections
NT = S // XT               # 8 token tiles per batch
QT = 512                   # q-tile width for attention
NQT = S // QT              # 4 q-tiles
ET = 512                   # e-tile width for out-projection
SCALE = 1.0 / float(np.sqrt(D))
MASK_BIAS = -30.0


def build_nc():
    nc = bacc.Bacc(trn_type="TRN2", target_bir_lowering=False, num_swdge_queues=4)

    xT = nc.declare_dram_parameter("xT", [B, E, S], F16, isOutput=False)
    wq = nc.declare_dram_parameter("wq", [E, M], F16, isOutput=False)
    wk = nc.declare_dram_parameter("wk", [E, M], F16, isOutput=False)
    wv = nc.declare_dram_parameter("wv", [E, M], F16, isOutput=False)
    wo = nc.declare_dram_parameter("wo", [M, E], F16, isOutput=False)
    bq = nc.declare_dram_parameter("bq", [128, HPC], F32, isOutput=False)
    bk = nc.declare_dram_parameter("bk", [128, HPC], F32, isOutput=False)
    tb = nc.declare_dram_parameter("tb", [128, 128], F32, isOutput=False)
    ident = nc.declare_dram_parameter("ident", [128, 128], F16, isOutput=False)
    o = nc.declare_dram_parameter("o", [B, S, E], F16, isOutput=True)

    with tile.TileContext(nc) as tc:
        _body(tc, nc, xT, wq, wk, wv, wo, bq, bk, tb, ident, o)
    nc.compile()
    return nc


def _body(tc, nc, xT, wq, wk, wv, wo, bq, bk, tb, ident, o):
    from contextlib import ExitStack

    ctx = ExitStack()
    with ctx:
        wpool = ctx.enter_context(tc.tile_pool(name="w", bufs=1))
        xpool = ctx.enter_context(tc.tile_pool(name="x", bufs=3))
        qkv = ctx.enter_context(tc.tile_pool(name="qkv", bufs=2))
        epool = ctx.enter_context(tc.tile_pool(name="e", bufs=6))
        otp = ctx.enter_context(tc.tile_pool(name="ot", bufs=2))
        osp = ctx.enter_context(tc.tile_pool(name="os", bufs=8))
        rp = ctx.enter_context(tc.tile_pool(name="r", bufs=2))
        psA = ctx.enter_context(tc.tile_pool(name="psA", bufs=2, space="PSUM"))
        psC = ctx.enter_context(tc.tile_pool(name="psC", bufs=3, space="PSUM"))
        psB = ctx.enter_context(tc.tile_pool(name="psB", bufs=2, space="PSUM"))
        psZ = ctx.enter_context(tc.tile_pool(name="psZ", bufs=1, space="PSUM"))

        # ---- weights / constants (once) ----
        wq_sb = wpool.tile([128, EO, M], F16, tag="wq")
        wk_sb = wpool.tile([128, EO, M], F16, tag="wk")
        wv_sb = wpool.tile([128, EO, M], F16, tag="wv")
        wo_sb = wpool.tile([128, HPC, E], F16, tag="wo")
        on_sb = wpool.tile([128, 1], F16, tag="on")
        nc.vector.memset(on_sb[:], 1.0)
        # Warm the PE (p-state ramp) with matmuls while x0/weights load.
        warm_rhs = rp.tile([128, 512], F16, tag="wr")
        nc.vector.memset(warm_rhs[:], 0.0)
        warm = psC.tile([128, 512], F32, tag="sc")
        for _ in range(24):
            nc.tensor.matmul(warm[:1, :], on_sb[:], warm_rhs[:],
                             start=True, stop=True)
        # x tile 0 and wq stream in interleaved chunks so the first projection
        # matmuls start as soon as their first contraction chunks land; wo is
        # not needed until attention output, so it loads last.
        x_first = xpool.tile([128, EO, XT], F16, tag="x")
        _xr0 = xT[0].rearrange("(eo p) s -> p eo s", p=128)
        _wqr = wq.rearrange("(eo p) m -> p eo m", p=128)
        for lo, hi in ((0, 4), (4, 8), (8, 16)):
            nc.gpsimd.dma_start(x_first[:, lo:hi], _xr0[:, lo:hi, 0:XT])
            nc.gpsimd.dma_start(wq_sb[:, lo:hi], _wqr[:, lo:hi])
        nc.gpsimd.dma_start(wk_sb[:], wk.rearrange("(eo p) m -> p eo m", p=128))
        nc.gpsimd.dma_start(wv_sb[:], wv.rearrange("(eo p) m -> p eo m", p=128))
        x_second = xpool.tile([128, EO, XT], F16, tag="x")
        nc.gpsimd.dma_start(x_second[:], _xr0[:, :, XT:2 * XT])
        nc.gpsimd.dma_start(wo_sb[:], wo.rearrange("(h p) e -> p h e", p=128))
        bq_sb = wpool.tile([128, HPC], F32, tag="bq")
        bk_sb = wpool.tile([128, HPC], F32, tag="bk")
        nc.sync.dma_start(bq_sb[:], bq[:])
        nc.sync.dma_start(bk_sb[:], bk[:])
        tb_sb = wpool.tile([128, 128], F32, tag="tb")
        nc.sync.dma_start(tb_sb[:], tb[:])
        id_sb = wpool.tile([128, 128], F16, tag="id")
        nc.sync.dma_start(id_sb[:], ident[:])

        # per-batch state (filled by emit_proj_*, read by attention)
        st = {}

        def new_batch(b):
            qT_sb = qkv.tile([128, HPC, S], F16, tag="qT")
            kT_sb = qkv.tile([128, HPC, S], F16, tag="kT")
            v_sb = qkv.tile([128, S // 128, M], F16, tag="v")
            oT_sb = otp.tile([128, HPC, S], F16, tag="oT")
            st[b] = dict(
                qT_h=[qT_sb[:, h] for h in range(HPC)],
                kT_h=[kT_sb[:, h] for h in range(HPC)],
                v_sb=v_sb,
                v_h=[v_sb[:, :, h * D:(h + 1) * D] for h in range(HPC)],
                oT_h=[oT_sb[:, h] for h in range(HPC)],
            )

        def proj_tile(b, t, x_t):
            sb = st[b]
            for h in range(HPC):
                for w_sb, dsts, bias, scl in (
                    (wq_sb, sb["qT_h"], bq_sb, SCALE),
                    (wk_sb, sb["kT_h"], bk_sb, 1.0),
                ):
                    ps = psA.tile([128, 512], F32, tag="qkv")
                    for eo in range(EO):
                        nc.tensor.matmul(
                            ps[:, :XT],
                            w_sb[:, eo, h * D:(h + 1) * D],
                            x_t[:, eo, :],
                            start=(eo == 0),
                            stop=(eo == EO - 1),
                        )
                    nc.scalar.activation(
                        dsts[h][:, t * XT:(t + 1) * XT],
                        ps[:, :XT],
                        AF.Identity,
                        bias=bias[:, h:h + 1],
                        scale=scl,
                    )
            for sti in range(XT // 128):
                ps = psA.tile([128, 512], F32, tag="qkv")
                for eo in range(EO):
                    nc.tensor.matmul(
                        ps[:, :M],
                        x_t[:, eo, sti * 128:(sti + 1) * 128],
                        wv_sb[:, eo, :],
                        start=(eo == 0),
                        stop=(eo == EO - 1),
                    )
                nc.vector.tensor_copy(
                    sb["v_sb"][:, t * (XT // 128) + sti, :], ps[:, :M]
                )

        def attn_qtile(b, h, qt):
            """Emit the k-loop (scores/exp with av+Z lagged 2 k-tiles so the
            PE wait-queue never fills on a pending exp) plus the reciprocal;
            returns a closure that emits the chain tail (transpose →
            broadcast → normalize), to be called once later PE work can
            cover its latency."""
            sb = st[b]
            q_rhs = sb["qT_h"][h][:, qt * QT:(qt + 1) * QT]
            ut = psB.tile([128, 512], F32, tag="ut")
            # Z accumulator: chunk qi lives at column qi*4 (16-byte spacing —
            # PSUM matmul outputs at unaligned 4-byte offsets misbehave)
            zt = psZ.tile([128, 512], F32, tag="z")
            nkt = (qt + 1) * (QT // 128)
            pend = []

            def flush(keep):
                while len(pend) > keep:
                    kt0, e0, lo0 = pend.pop(0)
                    nc.tensor.matmul(
                        ut[:, lo0:],
                        sb["v_h"][h][:, kt0, :],
                        e0[:, lo0:],
                        start=(kt0 == 0),
                        stop=(kt0 == nkt - 1),
                    )
                    # Z accumulation: per 128-wide q-chunk, a transposed
                    # tiny matmul (output [128,1], ~free on the PE) sums e
                    # over this k-tile's 128 keys, accumulating in PSUM.
                    # PSUM start/stop are bank-granular (2KB zero-region):
                    # exactly one start on the bank's first write (arms
                    # pending-zero for every byte, so each column's first
                    # write SETs) and one stop on the very last write.
                    for qi in range(lo0 // 128, 4):
                        nc.tensor.matmul(
                            zt[:, qi * 4:qi * 4 + 1],
                            e0[:, qi * 128:(qi + 1) * 128],
                            on_sb[:],
                            start=(kt0 == 0 and qi == 0),
                            stop=(kt0 == nkt - 1 and qi == 3),
                        )

            for kt in range(nkt):
                jj = kt - qt * (QT // 128)
                # columns < jj*128 of this k-tile's block are causally
                # masked; diagonal tiles come last in the k-loop, so
                # accumulating only the valid sub-range is exact.
                lo = max(jj, 0) * 128
                sc = psC.tile([128, 512], F32, tag="sc")
                nc.tensor.matmul(
                    sc[:, lo:],
                    sb["kT_h"][h][:, kt * 128:(kt + 1) * 128],
                    q_rhs[:, lo:],
                    start=True,
                    stop=True,
                )
                e = epool.tile([128, 512], F16, tag="e")
                if jj >= 0:
                    nc.vector.tensor_tensor(
                        sc[:, jj * 128:(jj + 1) * 128],
                        sc[:, jj * 128:(jj + 1) * 128],
                        tb_sb[:],
                        ALU.add,
                    )
                nc.scalar.activation(e[:, lo:], sc[:, lo:], AF.Exp)
                pend.append((kt, e, lo))
                flush(2)
            flush(0)
            rc4 = rp.tile([128, 16], F16, tag="rc4")
            with nc.allow_low_precision(reason="1/Z fp16 rel err ~5e-4"):
                nc.vector.reciprocal(rc4[:], zt[:, 0:16])

            def chain():
                # broadcast-transpose 1/Z from q-partition columns back to a
                # row: rc_col^T @ I gives a [1,128] row at partition 0 (fp16
                # matmul, 53ns each).  Scratch lives in a borrowed scores
                # slot so it can't clobber the next head's Z accumulator.
                zr = psC.tile([128, 512], F32, tag="sc")
                for qi in range(4):
                    nc.tensor.matmul(
                        zr[0:1, qi * 128:(qi + 1) * 128],
                        rc4[:, qi * 4:qi * 4 + 1],
                        id_sb[:],
                        start=(qi == 0),
                        stop=(qi == 3),
                    )
                rec_row = rp.tile([1, 512], F32, tag="rec")
                nc.vector.tensor_copy(rec_row[:], zr[0:1, :])
                rb = rp.tile([128, 512], F32, tag="rb")
                nc.gpsimd.partition_broadcast(rb[:], rec_row[:])
                nc.vector.tensor_tensor(
                    sb["oT_h"][h][:, qt * QT:(qt + 1) * QT], ut[:], rb[:],
                    ALU.mult
                )

            return chain

        def out_proj(b, qt, spread=False):
            sb = st[b]
            for qi4 in range(QT // 128):
                qi = qt * (QT // 128) + qi4
                for et in range(E // ET):
                    ps = psA.tile([128, 512], F32, tag="qkv")
                    for h in range(HPC):
                        nc.tensor.matmul(
                            ps[:],
                            sb["oT_h"][h][:, qi * 128:(qi + 1) * 128],
                            wo_sb[:, h, et * ET:(et + 1) * ET],
                            start=(h == 0),
                            stop=(h == HPC - 1),
                        )
                    osb = osp.tile([128, 512], F16, tag="osb")
                    r = (qi4 * (E // ET) + et) % 4
                    if r == 1:
                        nc.scalar.copy(osb[:], ps[:])
                    else:
                        nc.vector.tensor_copy(osb[:], ps[:])
                    # near the end of the kernel, spread output DMAs over
                    # several DGE queues so the drain isn't serialized on SP
                    eng = (
                        (nc.sync, nc.scalar, nc.gpsimd)[(qi4 * 4 + et) % 3]
                        if spread else nc.sync
                    )
                    eng.dma_start(
                        o[b, qi * 128:(qi + 1) * 128, et * ET:(et + 1) * ET],
                        osb[:],
                    )

        xTr = [xT[b].rearrange("(eo p) s -> p eo s", p=128) for b in range(B)]

        def load_x(b, t):
            x_t = xpool.tile([128, EO, XT], F16, tag="x")
            nc.gpsimd.dma_start(x_t[:], xTr[b][:, :, t * XT:(t + 1) * XT])
            return x_t

        # ---- batch 0 projections ----
        new_batch(0)
        for t in range(NT):
            if t == 0:
                x_t = x_first
            elif t == 1:
                x_t = x_second
            else:
                x_t = load_x(0, t)
            proj_tile(0, t, x_t)

        # ---- batch 0 attention, interleaved with batch 1 projections ----
        new_batch(1)
        xq = [load_x(1, 0)]  # prefetch queue for b1 x tiles

        for qt in range(NQT):
            c0 = attn_qtile(0, 0, qt)
            c1 = attn_qtile(0, 1, qt)
            c0()
            for i, t in enumerate((2 * qt, 2 * qt + 1)):
                if t + 1 < NT:
                    xq.append(load_x(1, t + 1))
                proj_tile(1, t, xq.pop(0))
                if i == 0:
                    c1()
            out_proj(0, qt)

        # ---- batch 1 attention, out-projections deferred one q-tile so the
        # softmax chains always have PE work covering their latency ----
        for qt in range(NQT):
            c0 = attn_qtile(1, 0, qt)
            c1 = attn_qtile(1, 1, qt)
            c0()
            if qt < NQT - 1:
                if qt >= 1:
                    out_proj(1, qt - 1)
                c1()
            else:
                c1()
                out_proj(1, qt - 1, spread=True)
                out_proj(1, qt, spread=True)


_NC_CACHE = None


def _get_nc():
    global _NC_CACHE
    if _NC_CACHE is None:
        _NC_CACHE = build_nc()
    return _NC_CACHE


def _prep_inputs(x, Wq, bq, Wk, bk, Wv, bv, Wo, bo):
    x = np.asarray(x, dtype=np.float32)
    xT = np.ascontiguousarray(x.transpose(0, 2, 1)).astype(np.float16)
    tb_np = np.where(
        np.arange(128)[:, None] <= np.arange(128)[None, :], 0.0, MASK_BIAS
    ).astype(np.float32)
    id_np = np.eye(128, dtype=np.float16)
    in_maps = []
    for c in range(NCORES):
        sl = slice(c * M, (c + 1) * M)
        in_maps.append({
            "xT": xT,
            "wq": np.ascontiguousarray(np.asarray(Wq)[sl, :].T.astype(np.float16)),
            "wk": np.ascontiguousarray(np.asarray(Wk)[sl, :].T.astype(np.float16)),
            "wv": np.ascontiguousarray(np.asarray(Wv)[sl, :].T.astype(np.float16)),
            "wo": np.ascontiguousarray(np.asarray(Wo)[:, sl].T.astype(np.float16)),
            "bq": np.ascontiguousarray(
                (np.asarray(bq)[sl].astype(np.float32) * SCALE).reshape(HPC, 128).T
            ),
            "bk": np.ascontiguousarray(
                np.asarray(bk)[sl].astype(np.float32).reshape(HPC, 128).T
            ),
            "tb": tb_np,
            "ident": id_np,
        })
    return in_maps


def run(inputs, trace=False):
    in_maps = _prep_inputs(
        inputs["x"], inputs["Wq"], inputs["bq"], inputs["Wk"], inputs["bk"],
        inputs["Wv"], inputs["bv"], inputs["Wo"], inputs["bo"],
    )
    nc = _get_nc()
    res = run_bass_kernel_spmd(nc, in_maps, list(range(NCORES)), trace=trace)
    acc = np.zeros((B, S, E), dtype=np.float64)
    for r in res.results:
        acc += r["o"].astype(np.float64)
    acc += np.asarray(inputs["bo"], dtype=np.float64)[None, None, :]
    acc += (np.asarray(inputs["bv"], dtype=np.float64)
            @ np.asarray(inputs["Wo"], dtype=np.float64).T)[None, None, :]
    return acc.astype(np.float32), res


def kernel(**inputs):
    out, _ = run(inputs, trace=False)
    return out


# revision 30
# speedup vs baseline: 1.1506x; 1.0065x over previous
"""Multi-head causal attention on 8 TRN2 NeuronCores.

Sharding: tensor-parallel over heads — 16 heads / 8 cores = 2 heads per core.
Each core computes q/k/v projections for its 2 heads (column-sharded QKV
weights), causal attention for those heads over both batch elements, and the
row-sharded slice of the output projection, producing a full-shape partial
output.  Host sums the 8 partials and adds bo + bv @ Wo.T (the per-head value
bias commutes through the output projection because attention rows sum to 1).

v2 layout/structure choices (vs the fp32r baseline):
  - fp16 operands everywhere (x, weights, q/k/v, e, oT, output partials);
    PSUM accumulation stays fp32.  fp16 matmuls run 1 cycle/row at any
    width (fp32r needs >=256-wide), halving DMA and SBUF on the way.
  - softmax denominators are NOT computed with ones-row matmuls (those cost
    a full PE pass per e-tile).  Instead, per 128-wide q-chunk, a tiny
    transposed matmul (lhsT = e-chunk, rhs = ones column, output [128,1])
    accumulates Z in PSUM across k-tiles at ~zero PE cost, directly in
    q-partition layout.  One cheap PE transpose per (h,qt) brings 1/Z back
    to row layout for the broadcast + normalization multiply.
  - scores are computed transposed [k, q]; diagonal k-tiles only compute
    their causally valid column sub-range (exact: diagonal tiles are last)
  - engines execute their instruction streams in order, so batch-1
    projections are EMITTED interleaved into batch-0's attention loop to
    fill PE stalls; out-projections in batch 1 are deferred by one q-tile
    to hide the softmax normalization chain.
  - out-projection evacuations round-robin over DVE/ACT/gpsimd.
"""

import sys

if "/opt/trn_rl_repo" not in sys.path:
    sys.path.insert(0, "/opt/trn_rl_repo")

import numpy as np

import concourse.bass as bass  # noqa: F401  (engine namespaces live on nc)
import concourse.tile as tile
from concourse import bacc, mybir
from concourse.bass_utils import run_bass_kernel_spmd

F32 = mybir.dt.float32
F32R = mybir.dt.float32r
F16 = mybir.dt.float16
AF = mybir.ActivationFunctionType
ALU = mybir.AluOpType

B, S, E = 2, 2048, 2048
H, D = 16, 128
NCORES = 8
HPC = H // NCORES          # heads per core = 2
M = HPC * D                # local channels per core = 256
EO = E // 128              # 16 contraction chunks
XT = 256                   # token-tile width for projections
NT = S // XT               # 8 token tiles per batch
QT = 512                   # q-tile width for attention
NQT = S // QT              # 4 q-tiles
ET = 512                   # e-tile width for out-projection
SCALE = 1.0 / float(np.sqrt(D))
MASK_BIAS = -30.0


def build_nc():
    nc = bacc.Bacc(trn_type="TRN2", target_bir_lowering=False, num_swdge_queues=4)

    xT = nc.declare_dram_parameter("xT", [B, E, S], F16, isOutput=False)
    wq = nc.declare_dram_parameter("wq", [E, M], F16, isOutput=False)
    wk = nc.declare_dram_parameter("wk", [E, M], F16, isOutput=False)
    wv = nc.declare_dram_parameter("wv", [E, M], F16, isOutput=False)
    wo = nc.declare_dram_parameter("wo", [M, E], F16, isOutput=False)
    bq = nc.declare_dram_parameter("bq", [128, HPC], F32, isOutput=False)
    bk = nc.declare_dram_parameter("bk", [128, HPC], F32, isOutput=False)
    tb = nc.declare_dram_parameter("tb", [128, 128], F32, isOutput=False)
    ident = nc.declare_dram_parameter("ident", [128, 128], F16, isOutput=False)
    o = nc.declare_dram_parameter("o", [B, S, E], F16, isOutput=True)

    with tile.TileContext(nc) as tc:
        _body(tc, nc, xT, wq, wk, wv, wo, bq, bk, tb, ident, o)
    nc.compile()
    return nc


def _body(tc, nc, xT, wq, wk, wv, wo, bq, bk, tb, ident, o):
    from contextlib import ExitStack

    ctx = ExitStack()
    with ctx:
        wpool = ctx.enter_context(tc.tile_pool(name="w", bufs=1))
        xpool = ctx.enter_context(tc.tile_pool(name="x", bufs=3))
        qkv = ctx.enter_context(tc.tile_pool(name="qkv", bufs=2))
        epool = ctx.enter_context(tc.tile_pool(name="e", bufs=6))
        otp = ctx.enter_context(tc.tile_pool(name="ot", bufs=2))
        osp = ctx.enter_context(tc.tile_pool(name="os", bufs=8))
        rp = ctx.enter_context(tc.tile_pool(name="r", bufs=2))
        psA = ctx.enter_context(tc.tile_pool(name="psA", bufs=2, space="PSUM"))
        psC = ctx.enter_context(tc.tile_pool(name="psC", bufs=3, space="PSUM"))
        psB = ctx.enter_context(tc.tile_pool(name="psB", bufs=2, space="PSUM"))
        psZ = ctx.enter_context(tc.tile_pool(name="psZ", bufs=1, space="PSUM"))

        # ---- weights / constants (once) ----
        wq_sb = wpool.tile([128, EO, M], F16, tag="wq")
        wk_sb = wpool.tile([128, EO, M], F16, tag="wk")
        wv_sb = wpool.tile([128, EO, M], F16, tag="wv")
        wo_sb = wpool.tile([128, HPC, E], F16, tag="wo")
        on_sb = wpool.tile([128, 1], F16, tag="on")
        nc.vector.memset(on_sb[:], 1.0)
        # Warm the PE (p-state ramp) with matmuls while x0/weights load.
        warm_rhs = rp.tile([128, 512], F16, tag="wr")
        nc.vector.memset(warm_rhs[:], 0.0)
        warm = psC.tile([128, 512], F32, tag="sc")
        for _ in range(24):
            nc.tensor.matmul(warm[:1, :], on_sb[:], warm_rhs[:],
                             start=True, stop=True)
        # x tile 0 and wq stream in interleaved chunks so the first projection
        # matmuls start as soon as their first contraction chunks land; wo is
        # not needed until attention output, so it loads last.
        x_first = xpool.tile([128, EO, XT], F16, tag="x")
        _xr0 = xT[0].rearrange("(eo p) s -> p eo s", p=128)
        _wqr = wq.rearrange("(eo p) m -> p eo m", p=128)
        for lo, hi in ((0, 4), (4, 8), (8, 16)):
            nc.sync.dma_start(x_first[:, lo:hi], _xr0[:, lo:hi, 0:XT])
            nc.scalar.dma_start(wq_sb[:, lo:hi], _wqr[:, lo:hi])
        nc.scalar.dma_start(wk_sb[:], wk.rearrange("(eo p) m -> p eo m", p=128))
        nc.scalar.dma_start(wv_sb[:], wv.rearrange("(eo p) m -> p eo m", p=128))
        x_second = xpool.tile([128, EO, XT], F16, tag="x")
        nc.sync.dma_start(x_second[:], _xr0[:, :, XT:2 * XT])
        nc.scalar.dma_start(wo_sb[:], wo.rearrange("(h p) e -> p h e", p=128))
        bq_sb = wpool.tile([128, HPC], F32, tag="bq")
        bk_sb = wpool.tile([128, HPC], F32, tag="bk")
        nc.sync.dma_start(bq_sb[:], bq[:])
        nc.sync.dma_start(bk_sb[:], bk[:])
        tb_sb = wpool.tile([128, 128], F32, tag="tb")
        nc.sync.dma_start(tb_sb[:], tb[:])
        id_sb = wpool.tile([128, 128], F16, tag="id")
        nc.sync.dma_start(id_sb[:], ident[:])

        # per-batch state (filled by emit_proj_*, read by attention)
        st = {}

        def new_batch(b):
            qT_sb = qkv.tile([128, HPC, S], F16, tag="qT")
            kT_sb = qkv.tile([128, HPC, S], F16, tag="kT")
            v_sb = qkv.tile([128, S // 128, M], F16, tag="v")
            oT_sb = otp.tile([128, HPC, S], F16, tag="oT")
            st[b] = dict(
                qT_h=[qT_sb[:, h] for h in range(HPC)],
                kT_h=[kT_sb[:, h] for h in range(HPC)],
                v_sb=v_sb,
                v_h=[v_sb[:, :, h * D:(h + 1) * D] for h in range(HPC)],
                oT_h=[oT_sb[:, h] for h in range(HPC)],
            )

        def proj_tile(b, t, x_t):
            sb = st[b]
            for h in range(HPC):
                for w_sb, dsts, bias, scl in (
                    (wq_sb, sb["qT_h"], bq_sb, SCALE),
                    (wk_sb, sb["kT_h"], bk_sb, 1.0),
                ):
                    ps = psA.tile([128, 512], F32, tag="qkv")
                    for eo in range(EO):
                        nc.tensor.matmul(
                            ps[:, :XT],
                            w_sb[:, eo, h * D:(h + 1) * D],
                            x_t[:, eo, :],
                            start=(eo == 0),
                            stop=(eo == EO - 1),
                        )
                    nc.scalar.activation(
                        dsts[h][:, t * XT:(t + 1) * XT],
                        ps[:, :XT],
                        AF.Identity,
                        bias=bias[:, h:h + 1],
                        scale=scl,
                    )
            for sti in range(XT // 128):
                ps = psA.tile([128, 512], F32, tag="qkv")
                for eo in range(EO):
                    nc.tensor.matmul(
                        ps[:, :M],
                        x_t[:, eo, sti * 128:(sti + 1) * 128],
                        wv_sb[:, eo, :],
                        start=(eo == 0),
                        stop=(eo == EO - 1),
                    )
                nc.vector.tensor_copy(
                    sb["v_sb"][:, t * (XT // 128) + sti, :], ps[:, :M]
                )

        def attn_qtile(b, h, qt):
            """Emit the k-loop (scores/exp with av+Z lagged 2 k-tiles so the
            PE wait-queue never fills on a pending exp) plus the reciprocal;
            returns a closure that emits the chain tail (transpose →
            broadcast → normalize), to be called once later PE work can
            cover its latency."""
            sb = st[b]
            q_rhs = sb["qT_h"][h][:, qt * QT:(qt + 1) * QT]
            ut = psB.tile([128, 512], F32, tag="ut")
            # Z accumulator: chunk qi lives at column qi*4 (16-byte spacing —
            # PSUM matmul outputs at unaligned 4-byte offsets misbehave)
            zt = psZ.tile([128, 512], F32, tag="z")
            nkt = (qt + 1) * (QT // 128)
            pend = []

            def flush(keep):
                while len(pend) > keep:
                    kt0, e0, lo0 = pend.pop(0)
                    nc.tensor.matmul(
                        ut[:, lo0:],
                        sb["v_h"][h][:, kt0, :],
                        e0[:, lo0:],
                        start=(kt0 == 0),
                        stop=(kt0 == nkt - 1),
                    )
                    # Z accumulation: per 128-wide q-chunk, a transposed
                    # tiny matmul (output [128,1], ~free on the PE) sums e
                    # over this k-tile's 128 keys, accumulating in PSUM.
                    # PSUM start/stop are bank-granular (2KB zero-region):
                    # exactly one start on the bank's first write (arms
                    # pending-zero for every byte, so each column's first
                    # write SETs) and one stop on the very last write.
                    for qi in range(lo0 // 128, 4):
                        nc.tensor.matmul(
                            zt[:, qi * 4:qi * 4 + 1],
                            e0[:, qi * 128:(qi + 1) * 128],
                            on_sb[:],
                            start=(kt0 == 0 and qi == 0),
                            stop=(kt0 == nkt - 1 and qi == 3),
                        )

            for kt in range(nkt):
                jj = kt - qt * (QT // 128)
                # columns < jj*128 of this k-tile's block are causally
                # masked; diagonal tiles come last in the k-loop, so
                # accumulating only the valid sub-range is exact.
                lo = max(jj, 0) * 128
                sc = psC.tile([128, 512], F32, tag="sc")
                nc.tensor.matmul(
                    sc[:, lo:],
                    sb["kT_h"][h][:, kt * 128:(kt + 1) * 128],
                    q_rhs[:, lo:],
                    start=True,
                    stop=True,
                )
                e = epool.tile([128, 512], F16, tag="e")
                if jj >= 0:
                    nc.vector.tensor_tensor(
                        sc[:, jj * 128:(jj + 1) * 128],
                        sc[:, jj * 128:(jj + 1) * 128],
                        tb_sb[:],
                        ALU.add,
                    )
                nc.scalar.activation(e[:, lo:], sc[:, lo:], AF.Exp)
                pend.append((kt, e, lo))
                flush(2)
            flush(0)
            rc4 = rp.tile([128, 16], F16, tag="rc4")
            with nc.allow_low_precision(reason="1/Z fp16 rel err ~5e-4"):
                nc.vector.reciprocal(rc4[:], zt[:, 0:16])

            rbbox = {}

            def chain_a():
                # broadcast-transpose 1/Z from q-partition columns back to a
                # row: rc_col^T @ I gives a [1,128] row at partition 0 (fp16
                # matmul, 53ns each).  Scratch lives in a borrowed scores
                # slot so it can't clobber the next head's Z accumulator.
                zr = psC.tile([128, 512], F32, tag="sc")
                for qi in range(4):
                    nc.tensor.matmul(
                        zr[0:1, qi * 128:(qi + 1) * 128],
                        rc4[:, qi * 4:qi * 4 + 1],
                        id_sb[:],
                        start=(qi == 0),
                        stop=(qi == 3),
                    )
                rec_row = rp.tile([1, 512], F32, tag="rec")
                nc.vector.tensor_copy(rec_row[:], zr[0:1, :])
                rb = rp.tile([128, 512], F32, tag="rb")
                nc.gpsimd.partition_broadcast(rb[:], rec_row[:])
                rbbox["rb"] = rb

            def chain_b():
                # emitted after other DVE work so the mult (which waits on
                # the gpsimd broadcast) never head-of-line blocks evacs
                nc.vector.tensor_tensor(
                    sb["oT_h"][h][:, qt * QT:(qt + 1) * QT], ut[:],
                    rbbox["rb"][:], ALU.mult
                )

            return chain_a, chain_b

        def out_proj(b, qt, spread=False):
            sb = st[b]
            for qi4 in range(QT // 128):
                qi = qt * (QT // 128) + qi4
                for et in range(E // ET):
                    ps = psA.tile([128, 512], F32, tag="qkv")
                    for h in range(HPC):
                        nc.tensor.matmul(
                            ps[:],
                            sb["oT_h"][h][:, qi * 128:(qi + 1) * 128],
                            wo_sb[:, h, et * ET:(et + 1) * ET],
                            start=(h == 0),
                            stop=(h == HPC - 1),
                        )
                    osb = osp.tile([128, 512], F16, tag="osb")
                    r = (qi4 * (E // ET) + et) % 3
                    if r == 1:
                        nc.scalar.copy(osb[:], ps[:])
                    else:
                        nc.vector.tensor_copy(osb[:], ps[:])
                    # near the end of the kernel, spread output DMAs over
                    # several DGE queues so the drain isn't serialized on SP
                    eng = (
                        (nc.sync, nc.scalar)[(qi4 * 4 + et) % 2]
                        if spread else nc.sync
                    )
                    eng.dma_start(
                        o[b, qi * 128:(qi + 1) * 128, et * ET:(et + 1) * ET],
                        osb[:],
                    )

        xTr = [xT[b].rearrange("(eo p) s -> p eo s", p=128) for b in range(B)]

        def load_x(b, t):
            x_t = xpool.tile([128, EO, XT], F16, tag="x")
            nc.sync.dma_start(x_t[:], xTr[b][:, :, t * XT:(t + 1) * XT])
            return x_t

        # ---- batch 0 projections ----
        new_batch(0)
        for t in range(NT):
            if t == 0:
                x_t = x_first
            elif t == 1:
                x_t = x_second
            else:
                x_t = load_x(0, t)
            proj_tile(0, t, x_t)

        # ---- batch 0 attention, interleaved with batch 1 projections ----
        new_batch(1)
        xq = [load_x(1, 0)]  # prefetch queue for b1 x tiles

        for qt in range(NQT):
            a0, b0f = attn_qtile(0, 0, qt)
            a1, b1f = attn_qtile(0, 1, qt)
            a0()
            t = 2 * qt
            if t + 1 < NT:
                xq.append(load_x(1, t + 1))
            proj_tile(1, t, xq.pop(0))
            a1()
            b0f()
            t = 2 * qt + 1
            if t + 1 < NT:
                xq.append(load_x(1, t + 1))
            proj_tile(1, t, xq.pop(0))
            b1f()
            out_proj(0, qt)

        # ---- batch 1 attention, out-projections deferred one q-tile so the
        # softmax chains always have PE work covering their latency ----
        for qt in range(NQT):
            a0, b0f = attn_qtile(1, 0, qt)
            a1, b1f = attn_qtile(1, 1, qt)
            a0()
            a1()
            if qt >= 1:
                out_proj(1, qt - 1, spread=(qt == NQT - 1))
            b0f()
            b1f()
        out_proj(1, NQT - 1, spread=True)


_NC_CACHE = None


def _get_nc():
    global _NC_CACHE
    if _NC_CACHE is None:
        _NC_CACHE = build_nc()
    return _NC_CACHE


def _prep_inputs(x, Wq, bq, Wk, bk, Wv, bv, Wo, bo):
    x = np.asarray(x, dtype=np.float32)
    xT = np.ascontiguousarray(x.transpose(0, 2, 1)).astype(np.float16)
    tb_np = np.where(
        np.arange(128)[:, None] <= np.arange(128)[None, :], 0.0, MASK_BIAS
    ).astype(np.float32)
    id_np = np.eye(128, dtype=np.float16)
    in_maps = []
    for c in range(NCORES):
        sl = slice(c * M, (c + 1) * M)
        in_maps.append({
            "xT": xT,
            "wq": np.ascontiguousarray(np.asarray(Wq)[sl, :].T.astype(np.float16)),
            "wk": np.ascontiguousarray(np.asarray(Wk)[sl, :].T.astype(np.float16)),
            "wv": np.ascontiguousarray(np.asarray(Wv)[sl, :].T.astype(np.float16)),
            "wo": np.ascontiguousarray(np.asarray(Wo)[:, sl].T.astype(np.float16)),
            "bq": np.ascontiguousarray(
                (np.asarray(bq)[sl].astype(np.float32) * SCALE).reshape(HPC, 128).T
            ),
            "bk": np.ascontiguousarray(
                np.asarray(bk)[sl].astype(np.float32).reshape(HPC, 128).T
            ),
            "tb": tb_np,
            "ident": id_np,
        })
    return in_maps


def run(inputs, trace=False):
    in_maps = _prep_inputs(
        inputs["x"], inputs["Wq"], inputs["bq"], inputs["Wk"], inputs["bk"],
        inputs["Wv"], inputs["bv"], inputs["Wo"], inputs["bo"],
    )
    nc = _get_nc()
    res = run_bass_kernel_spmd(nc, in_maps, list(range(NCORES)), trace=trace)
    acc = np.zeros((B, S, E), dtype=np.float64)
    for r in res.results:
        acc += r["o"].astype(np.float64)
    acc += np.asarray(inputs["bo"], dtype=np.float64)[None, None, :]
    acc += (np.asarray(inputs["bv"], dtype=np.float64)
            @ np.asarray(inputs["Wo"], dtype=np.float64).T)[None, None, :]
    return acc.astype(np.float32), res


def kernel(**inputs):
    out, _ = run(inputs, trace=False)
    return out
